# revision 1
# baseline (speedup 1.0000x reference)
"""Multi-head cross-attention (self-attention variant) on 8 Trainium2 NeuronCores.

Problem: x[1,4096,1024]; Wq/Wk/Wv[1024,1024] -> 16 heads x 64 dim; softmax(QK^T/8)V;
merge heads; @ Wo + bo -> [1,4096,1024].

Sharding: tensor-parallel over heads. Core k owns heads (2k, 2k+1) = inner cols
[128k : 128k+128]. Each core computes Q^T/K^T (in [dh, n] layout) and V for its
heads, runs flash-style attention entirely on-chip (scores never hit HBM,
softmax without max-subtraction: scores ~ N(0,1) so exp is safe in fp32), and
produces normalized head outputs O^T [128, 4096]. An AllToAll re-shards from
head-parallel to sequence-parallel: core k ends up with the full 1024-dim inner
activation for rows [512k : 512k+512], then applies the full Wo to just its row
slice. Host concatenates row slices and adds bo.

Matmuls use float32r (fp32 storage, relaxed-precision PE mode, 1 cycle/row at
N>=256 vs 4 for fp32) except the V projection (N=128, where fp32r has no
advantage).
"""
import numpy as np
from contextlib import ExitStack

N_CORES = 8
N = 4096          # sequence length
QD = 1024         # model dim
DH = 64           # head dim
HPC = 2           # heads per core
CPC = HPC * DH    # inner cols per core = 128
IC = 512          # i-chunk (query) size
NI = N // IC      # 8 chunks
JB = 128          # j-block (key) size
NJ = N // JB      # 32 blocks
SCALE = DH ** -0.5
VW = DH + 1       # V columns per head incl. ones column (65)
VBW = 2 * VW      # V block width for both heads (130)

_CACHE = {}


def _build(debug=False, repeat=1, single=False):
    from concourse import bacc, tile, mybir

    f32 = mybir.dt.float32
    fr = mybir.dt.float32r
    Exp = mybir.ActivationFunctionType.Exp

    nc = bacc.Bacc("TRN2", target_bir_lowering=False, debug=False,
                   enable_asserts=False, num_devices=1 if single else N_CORES)

    xt_d = nc.dram_tensor("xt", [QD, N], fr, kind="ExternalInput").ap()
    wq_d = nc.dram_tensor("wq", [QD, CPC], fr, kind="ExternalInput").ap()
    wk_d = nc.dram_tensor("wk", [QD, CPC], fr, kind="ExternalInput").ap()
    wv_d = nc.dram_tensor("wv", [QD, CPC], fr, kind="ExternalInput").ap()
    wo_d = nc.dram_tensor("wo", [QD, QD], fr, kind="ExternalInput").ap()
    y_d = nc.dram_tensor("y_out", [IC, QD], f32, kind="ExternalOutput").ap()
    if debug:
        qt_dbg = nc.dram_tensor("qt_dbg", [CPC, N], f32, kind="ExternalOutput").ap()
        kt_dbg = nc.dram_tensor("kt_dbg", [CPC, N], f32, kind="ExternalOutput").ap()
        v_dbg = nc.dram_tensor("v_dbg", [128, NJ * VBW], f32, kind="ExternalOutput").ap()
        a2a_dbg = nc.dram_tensor("a2a_dbg", [N_CORES * CPC, IC], f32, kind="ExternalOutput").ap()
        go_dbg = nc.dram_tensor("go_dbg", [128, 8 * IC], f32, kind="ExternalOutput").ap()

    with tile.TileContext(nc) as tc:
        with ExitStack() as ctx:
            sb = ctx.enter_context(tc.tile_pool(name="sb", bufs=1))
            xt_pool = ctx.enter_context(tc.tile_pool(name="xt", bufs=2))
            pt_pool = ctx.enter_context(tc.tile_pool(name="pt", bufs=3))
            ot_pool = ctx.enter_context(tc.tile_pool(name="ot", bufs=4))
            sm_pool = ctx.enter_context(tc.tile_pool(name="sm", bufs=4))
            y_pool = ctx.enter_context(tc.tile_pool(name="ysb", bufs=3))
            ps1 = ctx.enter_context(tc.tile_pool(name="ps1", bufs=4, space="PSUM"))
            ps2 = ctx.enter_context(tc.tile_pool(name="ps2", bufs=2, space="PSUM"))
            dram = ctx.enter_context(tc.tile_pool(name="dram", bufs=1, space="DRAM"))

            # --- static SBUF residents (per-chunk tiles so attention on
            # chunk 0 can start while later chunks are still projecting) ---
            qts = [sb.tile([CPC, IC], fr, name=f"qt{c}") for c in range(NI)]
            kts = [sb.tile([CPC, IC], fr, name=f"kt{c}") for c in range(NI)]
            vs = [sb.tile([128, 4 * VBW], fr, name=f"v{c}") for c in range(NI)]
            wq_sb = sb.tile([128, QD], fr)      # QD-tile t at cols 128t
            wk_sb = sb.tile([128, QD], fr)
            wv_sb = sb.tile([128, QD], fr)
            wo_sb = sb.tile([128, 8 * QD], fr)  # c-tile t at cols 1024t
            go_sb = sb.tile([128, 8 * IC], fr)  # gathered O^T c-tile t at cols 512t

            a2a_in = dram.tile([N_CORES * CPC, IC], fr)
            a2a_out = dram.tile([N_CORES * CPC, IC], fr)

            # weight loads
            for t in range(8):
                nc.sync.dma_start(out=wq_sb[:, 128 * t:128 * t + CPC],
                                  in_=wq_d[128 * t:128 * t + 128, :])
                nc.sync.dma_start(out=wk_sb[:, 128 * t:128 * t + CPC],
                                  in_=wk_d[128 * t:128 * t + 128, :])
                nc.sync.dma_start(out=wv_sb[:, 128 * t:128 * t + CPC],
                                  in_=wv_d[128 * t:128 * t + 128, :])
                nc.sync.dma_start(out=wo_sb[:, QD * t:QD * (t + 1)],
                                  in_=wo_d[128 * t:128 * t + 128, :])

            # ones columns of v tiles (cols 64 and 129 of each 130-wide block);
            # memset can't write float32r, so stage f32 ones and convert via DVE
            ones_sb = sb.tile([128, 4], f32)
            nc.vector.memset(ones_sb[:, :], 1.0)
            for c in range(NI):
                v3 = vs[c].rearrange("p (j w) -> p j w", w=VBW)
                nc.vector.tensor_copy(v3[:, :, DH:DH + 1], ones_sb[:, :])
                nc.vector.tensor_copy(v3[:, :, VBW - 1:VBW], ones_sb[:, :])

            for _rep in range(repeat):
                # --- phase 1: projections ---
                # One PSUM accumulation group per tile: matmul start=True clears the
                # whole bank, so groups must not share banks.
                for c in range(NI):
                    xts = []
                    for t in range(8):
                        xt_t = xt_pool.tile([128, IC], fr, name=f"xt_{t}", tag=f"xt{t}")
                        nc.sync.dma_start(
                            out=xt_t[:, :],
                            in_=xt_d[128 * t:128 * t + 128, IC * c:IC * (c + 1)])
                        xts.append(xt_t)
                    q_ps = ps1.tile([128, IC], f32, tag="ps1", name="q_ps")
                    k_ps = ps1.tile([128, IC], f32, tag="ps1", name="k_ps")
                    for t in range(8):
                        st = dict(start=(t == 0), stop=(t == 7))
                        nc.tensor.matmul(q_ps[:, :], wq_sb[:, 128 * t:128 * t + CPC],
                                         xts[t][:, :], **st)
                        nc.tensor.matmul(k_ps[:, :], wk_sb[:, 128 * t:128 * t + CPC],
                                         xts[t][:, :], **st)
                    nc.vector.tensor_copy(qts[c][:, :], q_ps[:, :])
                    nc.vector.tensor_copy(kts[c][:, :], k_ps[:, :])
                    for b in range(4):
                        v_ps = ps1.tile([128, CPC], f32, tag="ps1", name="v_ps")
                        for t in range(8):
                            nc.tensor.matmul(
                                v_ps[:, :],
                                xts[t][:, 128 * b:128 * b + 128],
                                wv_sb[:, 128 * t:128 * t + CPC],
                                start=(t == 0), stop=(t == 7))
                        for h in range(HPC):
                            nc.vector.tensor_copy(
                                vs[c][:, VBW * b + VW * h:VBW * b + VW * h + DH],
                                v_ps[:, DH * h:DH * (h + 1)])

                # --- phase 2: attention (per chunk, per head) ---
                for c in range(NI):
                    for h in range(HPC):
                        hq = qts[c][DH * h:DH * (h + 1), :]
                        acc = ps1.tile([VW, IC], f32, tag="ps1", name="acc")
                        for g in range(NJ // 2):
                            s_ps = ps2.tile([128, 2 * IC], f32, name="s_ps")
                            pt = pt_pool.tile([128, 2 * IC], fr, name="pt")
                            for u in range(2):
                                jb = 2 * g + u
                                nc.tensor.matmul(
                                    s_ps[:, IC * u:IC * (u + 1)],
                                    kts[jb // 4][DH * h:DH * (h + 1),
                                                 JB * (jb % 4):JB * (jb % 4 + 1)],
                                    hq, start=True, stop=True)
                            nc.scalar.activation(pt[:, :], s_ps[:, :], Exp, scale=SCALE)
                            for u in range(2):
                                jb = 2 * g + u
                                nc.tensor.matmul(
                                    acc[:, :],
                                    vs[jb // 4][:, VBW * (jb % 4) + VW * h:
                                                VBW * (jb % 4) + VW * (h + 1)],
                                    pt[:, IC * u:IC * (u + 1)],
                                    start=(g == 0 and u == 0),
                                    stop=(g == NJ // 2 - 1 and u == 1))
                        # normalize: rows 0..63 are head out^T, row 64 is sum(exp)
                        rsum = sm_pool.tile([1, IC], f32, name="rsum")
                        nc.vector.tensor_copy(rsum[:, :], acc[DH:DH + 1, :])
                        rcp = sm_pool.tile([1, IC], f32, name="rcp")
                        nc.vector.reciprocal(rcp[:, :], rsum[:, :])
                        rb = sm_pool.tile([DH, IC], f32, name="rb")
                        nc.gpsimd.partition_broadcast(rb[:, :], rcp[:, :])
                        ot = ot_pool.tile([DH, IC], fr, name="ot")
                        nc.vector.tensor_mul(ot[:, :], acc[0:DH, :], rb[:, :])
                        row = CPC * c + DH * h
                        nc.sync.dma_start(out=a2a_in[row:row + DH, :], in_=ot[:, :])

                # --- phase 3: reshard + output projection ---
                if single:
                    nc.sync.dma_start(out=a2a_out[:, :], in_=a2a_in[:, :])
                else:
                    nc.gpsimd.collective_compute(
                        "AllToAll", mybir.AluOpType.bypass,
                        replica_groups=[list(range(N_CORES))],
                        ins=[a2a_in.opt()], outs=[a2a_out.opt()])
                for t in range(8):
                    nc.sync.dma_start(out=go_sb[:, IC * t:IC * (t + 1)],
                                      in_=a2a_out[128 * t:128 * t + 128, :])
                if debug:
                    nc.sync.dma_start(out=a2a_dbg[:, :], in_=a2a_in[:, :].bitcast(f32))
                    nc.sync.dma_start(out=go_dbg[:, :], in_=go_sb[:, :].bitcast(f32))
                for ib in range(IC // 128):
                    for e in range(2):
                        y_ps = ps1.tile([128, 512], f32, tag="ps1", name="y_ps")
                        for t in range(8):
                            nc.tensor.matmul(
                                y_ps[:, :],
                                go_sb[:, IC * t + 128 * ib:IC * t + 128 * (ib + 1)],
                                wo_sb[:, QD * t + 512 * e:QD * t + 512 * (e + 1)],
                                start=(t == 0), stop=(t == 7))
                        y_sb = y_pool.tile([128, 512], f32, name="y_sb")
                        nc.vector.tensor_copy(y_sb[:, :], y_ps[:, :])
                        nc.sync.dma_start(
                            out=y_d[128 * ib:128 * (ib + 1), 512 * e:512 * (e + 1)],
                            in_=y_sb[:, :])
    nc.compile()
    return nc


def _get_nc():
    if "nc" not in _CACHE:
        _CACHE["nc"] = _build()
    return _CACHE["nc"]


def kernel(x, Wq, Wk, Wv, Wo, bo):
    from concourse.bass_utils import run_bass_kernel_spmd

    x = np.asarray(x, dtype=np.float32)
    Wq = np.asarray(Wq, dtype=np.float32)
    Wk = np.asarray(Wk, dtype=np.float32)
    Wv = np.asarray(Wv, dtype=np.float32)
    Wo = np.asarray(Wo, dtype=np.float32)
    bo = np.asarray(bo, dtype=np.float32)

    nc = _get_nc()
    xt = np.ascontiguousarray(x.reshape(N, QD).T)
    in_maps = []
    for k in range(N_CORES):
        cs = CPC * k
        in_maps.append({
            "xt": xt,
            "wq": np.ascontiguousarray(Wq[:, cs:cs + CPC]),
            "wk": np.ascontiguousarray(Wk[:, cs:cs + CPC]),
            "wv": np.ascontiguousarray(Wv[:, cs:cs + CPC]),
            "wo": Wo,
        })
    res = run_bass_kernel_spmd(nc, in_maps, list(range(N_CORES)))
    y = np.concatenate([res.results[k]["y_out"] for k in range(N_CORES)], axis=0)
    y = y + bo[None, :]
    return y.reshape(1, N, QD).astype(np.float32)

